# revision 11
# baseline (speedup 1.0000x reference)
"""Trainium2 Bass kernel for causal multi-head attention with LoRA (QKV + proj).

Problem (hardcoded): B=4, T=2048, C=1024, NH=16, HD=64, RANK=56, alpha=8.

Sharding: tensor-parallel across heads — each of the 8 cores owns 2 heads
(128 qkv dims per projection) and processes all 4 batches. The output
projection is row-parallel (each core contracts over its own 128 y dims);
partial outputs are summed on the host.

v3 design notes:
- LoRA factors are folded into the dense weights on the host
  (W_eff = W + scaling*B@A — mathematically the same function).
- All matmuls fp16 (PSUM accumulation fp32); fp8 DoubleRow measured
  SLOWER than fp16 on this HW, so fp16 everywhere.
- ACT (scalar engine) costs ~1.4ns/col regardless of partition count or
  dtype, so ACT runs ONLY the softmax exp (~195us/core); every other
  PSUM->SBUF copy runs on DVE. Exp is the attention-phase bottleneck
  (> attention PE work), so attention is interleaved with other-phase PE
  work: during attn{b} the emission stream pulls "filler" units that run
  proj{b-1} and the QKV projection/V-transpose/K-pad prep of batch b+1
  on the otherwise-idle PE slots.
- Softmax normalize chain per (chunk, head) avoids GPSIMD/DMA partition
  moves entirely: DVE copies the sum row from PSUM, reciprocal_approx_fast,
  cast to fp16, then a K=1 matmul against a ones row broadcasts the
  per-column reciprocal across 64 partitions into PSUM; DVE multiplies
  psy (copied to SBUF fp16) by the broadcast. Head 1's PV stationary (vaB)
  stores V at block offset 64 (block stride 130, ones at offset 0..1) so
  its y lands on PSUM partitions 64..127 and its sums on partitions 0..1,
  keeping every engine op partition-base-aligned (engines cannot shift
  partitions; only DMA can, and DMA shifts cost ~600ns latency).
- PSUM budget (8 banks): mm=3 (QK scores + transposes + proj), acc=2
  (QKV accumulators), accy=2 (PV accumulators), bc=1 (recip broadcast).
- All big stationary operands are padded to full 128x128 tiles (zero-pad
  K, junk-tolerant PV slices) to stay on the PE fast-weight-load path.
"""
import os
import sys
import itertools
import numpy as np

if "/opt/trn_rl_repo" not in sys.path:
    sys.path.insert(0, "/opt/trn_rl_repo")

import concourse.bass as bass  # noqa: E402
from concourse import bacc  # noqa: E402
import concourse.mybir as mybir  # noqa: E402
import concourse.tile as tile  # noqa: E402
from concourse.bass_utils import run_bass_kernel_spmd  # noqa: E402

B, T, C = 4, 2048, 1024
NH, HD, RANK = 16, 64, 56
SCALING = 8.0 / 56.0
NCORES = 8
BT = B * T            # 8192
TOK = 512             # token chunk (matmul moving dim)
NT4 = T // TOK        # 4 token chunks per batch
NCIN = C // 128       # 8 input-feature chunks
NCO = C // 128        # 8 output-feature chunks (proj)
F32 = mybir.dt.float32
F16 = mybir.dt.float16
EXPF = mybir.ActivationFunctionType.Exp
COPYF = mybir.ActivationFunctionType.Identity

_cache = {}


def _build():
    nc = bacc.Bacc("TRN2", target_bir_lowering=False, debug=False,
                   num_devices=NCORES)
    xT = nc.dram_tensor("xT", [C, BT], F16, kind="ExternalInput")
    Wq = nc.dram_tensor("Wq", [NCIN, 128, 384], F16, kind="ExternalInput")
    bq = nc.dram_tensor("bq", [128, 3], F32, kind="ExternalInput")
    Wp = nc.dram_tensor("Wp", [128, C], F16, kind="ExternalInput")
    tri = nc.dram_tensor("tri", [128, 128], F16, kind="ExternalInput")
    eye = nc.dram_tensor("eye", [128, 128], F16, kind="ExternalInput")
    onesb = nc.dram_tensor("onesb", [128, 16], F16, kind="ExternalInput")
    zerosD = nc.dram_tensor("zerosD", [128, 1024], F16, kind="ExternalInput")
    outT = nc.dram_tensor("outT", [C, BT], F16, kind="ExternalOutput")
    DBG = os.environ.get("BASSDBG", "0") == "1"
    if DBG:
        zrd = nc.dram_tensor("zrd", [16 * 65, TOK], F32,
                             kind="ExternalOutput")
        r16d = nc.dram_tensor("r16d", [16 * 65, TOK], F16,
                              kind="ExternalOutput")
        ysd = nc.dram_tensor("ysd", [16 * 128, TOK], F16,
                             kind="ExternalOutput")

    with tile.TileContext(nc) as tc:
        with (
            tc.tile_pool(name="consts", bufs=1) as consts,
            tc.tile_pool(name="qkv", bufs=2) as qkvp,
            tc.tile_pool(name="persist", bufs=1) as persist,
            tc.tile_pool(name="ytp", bufs=2) as ytp,
            tc.tile_pool(name="xtp", bufs=16) as xtp,
            tc.tile_pool(name="expp", bufs=12) as expp,
            tc.tile_pool(name="small", bufs=2) as small,
            tc.tile_pool(name="ps", bufs=1, space="PSUM") as ps,
        ):
            wq_sb = consts.tile([128, NCIN, 384], F16)
            nc.sync.dma_start(wq_sb[:], Wq[:].rearrange("c p f -> p c f"))
            bias_sb = consts.tile([128, 3], F32)
            nc.sync.dma_start(bias_sb[:], bq[:])
            wp_sb = consts.tile([128, C], F16)
            nc.sync.dma_start(wp_sb[:], Wp[:])
            tri_sb = consts.tile([128, 128], F16)
            nc.sync.dma_start(tri_sb[:], tri[:])
            eye_sb = consts.tile([128, 128], F16)
            nc.sync.dma_start(eye_sb[:], eye[:])
            zeros_sb = consts.tile([128, TOK], F16)
            nc.sync.dma_start(zeros_sb[:], zerosD[:, 0:TOK])

            # persistent padded-K tiles and V-aug tiles, two sets
            # alternated by batch parity; zero/ones regions written once
            # through these same tile objects (so every later read is
            # dependency-tracked).
            kpset, vaset = {}, {}
            for par in range(2):
                for j in range(16):
                    for hh in range(2):
                        kpset[(par, j, hh)] = persist.tile(
                            [128, 128], F16, name=f"kp{par}_{j}_{hh}")
                for hh in range(2):
                    vaset[(par, hh)] = persist.tile(
                        [128, 16 * 66 + 64], F16, name=f"va{par}_{hh}")
            for par in range(2):
                for j in range(16):
                    nc.vector.tensor_copy(kpset[(par, j, 0)][64:128, :],
                                          zeros_sb[64:128, 0:128])
                    nc.vector.tensor_copy(kpset[(par, j, 1)][0:64, :],
                                          zeros_sb[0:64, 0:128])
                for hh in range(2):
                    va_ = vaset[(par, hh)]
                    nc.vector.tensor_copy(va_[:, 1056:1120],
                                          zeros_sb[:, 0:64])
                    vav_ = va_[:, 0:1056].rearrange("p (j c) -> p j c",
                                                    c=66)
                    for col in (64, 65):
                        nc.sync.dma_start(vav_[:, :, col:col + 1],
                                          onesb[:].unsqueeze(-1))

            # ---- per-batch persistent tiles ----
            def alloc_batch(b):
                st = {}
                st["qT"] = qkvp.tile([128, T], F16, tag="qT", name="qT")
                st["kT"] = qkvp.tile([128, T], F16, tag="kT", name="kT")
                st["vT"] = qkvp.tile([128, T], F16, tag="vT", name="vT")
                st["vaA"] = vaset[(b % 2, 0)]
                st["vaB"] = vaset[(b % 2, 1)]
                st["yt"] = ytp.tile([128, T], F16, tag="yt", name="yt")
                st["kps"] = {j: (kpset[(b % 2, j, 0)], kpset[(b % 2, j, 1)])
                             for j in range(16)}
                st["projq"] = []
                return st

            def gen_qkv(b, st):
                """QKV projection units for batch b (7 units per t4)."""
                qkvd = (st["qT"], st["kT"], st["vT"])
                for t4 in range(NT4):
                    hold = {}

                    def u_load(t4=t4, hold=hold):
                        gcol = b * T + t4 * TOK
                        xts = []
                        for cc in range(NCIN):
                            xt = xtp.tile([128, TOK], F16, tag="xt")
                            nc.sync.dma_start(
                                xt[:],
                                xT[cc * 128:(cc + 1) * 128,
                                   gcol:gcol + TOK])
                            xts.append(xt)
                        hold["x"] = xts
                    yield u_load
                    for ch in range(3):
                        def u_mm1(ch=ch, t4=t4, hold=hold):
                            ps_q = ps.tile([128, TOK], F32, tag="acc",
                                           bufs=2, name="psq")
                            hold[("p", ch)] = ps_q
                            for cc in range(4):
                                nc.tensor.matmul(
                                    ps_q[:],
                                    wq_sb[:, cc, ch * 128:(ch + 1) * 128],
                                    hold["x"][cc][:], start=(cc == 0),
                                    stop=False)

                        def u_mm2(ch=ch, t4=t4, hold=hold):
                            ps_q = hold.pop(("p", ch))
                            for cc in range(4, 8):
                                nc.tensor.matmul(
                                    ps_q[:],
                                    wq_sb[:, cc, ch * 128:(ch + 1) * 128],
                                    hold["x"][cc][:], start=False,
                                    stop=(cc == NCIN - 1))
                            nc.scalar.activation(
                                out=qkvd[ch][:, t4 * TOK:(t4 + 1) * TOK],
                                in_=ps_q[:], func=COPYF,
                                bias=bias_sb[:, ch:ch + 1], scale=1.0)
                        yield u_mm1
                        yield u_mm2

            def gen_vtr(b, st):
                """V -> token-major into vaA (h0) and vaB (h1, offset 64)."""
                vaA, vaB, vT = st["vaA"], st["vaB"], st["vT"]
                vaAv = vaA[:, 0:1056].rearrange("p (j c) -> p j c", c=66)
                vaBv = vaB[:, 0:1056].rearrange("p (j c) -> p j c", c=66)

                for tb in range(16):
                    def u(tb=tb):
                        ps_t = ps.tile([128, 128], F16, tag="mm", bufs=3)
                        nc.tensor.transpose(
                            ps_t[:], vT[:, tb * 128:(tb + 1) * 128],
                            eye_sb[:])
                        nc.vector.tensor_copy(vaAv[:, tb, 0:64],
                                              ps_t[:, 0:64])
                        nc.vector.tensor_copy(vaBv[:, tb, 0:64],
                                              ps_t[:, 64:128])
                    yield u

            def gen_kps(b, st):
                """Zero-padded per-head K tiles (head select via zeros)."""
                kT, kps = st["kT"], st["kps"]
                for j in range(16):
                    def u(j=j):
                        kp0, kp1 = kps[j]
                        nc.vector.tensor_copy(
                            kp0[0:64, :], kT[0:64, j * 128:(j + 1) * 128])
                        nc.vector.tensor_copy(
                            kp1[64:128, :],
                            kT[64:128, j * 128:(j + 1) * 128])
                    yield u

            def gen_proj_chunk(b, st, t4):
                """Output projection units for chunk t4 of batch b."""
                yt = st["yt"]
                if True:
                    for co in range(NCO):
                        def u(t4=t4, co=co):
                            gcol = b * T + t4 * TOK
                            ps_o = ps.tile([128, TOK], F32, tag="mm",
                                           bufs=3)
                            nc.tensor.matmul(
                                ps_o[:],
                                wp_sb[:, co * 128:(co + 1) * 128],
                                yt[:, t4 * TOK:(t4 + 1) * TOK],
                                start=True, stop=True)
                            po = small.tile([128, TOK], F16, tag="po",
                                            bufs=3)
                            nc.vector.tensor_copy(po[:], ps_o[:])
                            nc.sync.dma_start(
                                outT[co * 128:(co + 1) * 128,
                                     gcol:gcol + TOK], po[:])
                        yield u

            def emit_attn(b, st, fill):
                """Attention for batch b, pulling filler units into PE gaps."""
                qT, vaA, vaB, yt, kps = (st["qT"], st["vaA"], st["vaB"],
                                         st["yt"], st["kps"])
                pull = fill.pull

                for t4 in range(NT4):
                    nblk = 4 * (t4 + 1)
                    q0s, exps = {}, {}
                    psy0 = ps.tile([128, TOK], F32, tag="accy", bufs=3)
                    psy1 = ps.tile([128, TOK], F32, tag="accy", bufs=3)
                    psy = {0: psy0, 1: psy1}

                    def emit_qk(j, h, t4=t4, q0s=q0s, exps=exps, kps=kps):
                        r = j - 4 * t4
                        q0 = 128 * r if r > 0 else 0
                        q0s[j] = q0
                        ps_s = ps.tile([128, TOK], F32, tag="mm", bufs=3)
                        nc.tensor.matmul(
                            ps_s[:, q0:TOK],
                            kps[j][h][:],
                            qT[:, t4 * TOK + q0:(t4 + 1) * TOK],
                            start=True, stop=True)
                        e = expp.tile([128, TOK], F16, tag="expS")
                        nc.scalar.activation(
                            out=e[:, q0:TOK], in_=ps_s[:, q0:TOK],
                            func=EXPF, scale=0.125)
                        if r >= 0:
                            nc.vector.tensor_mul(
                                e[:, q0:q0 + 128], e[:, q0:q0 + 128],
                                tri_sb[:])
                        exps[(j, h)] = e

                    def emit_pv(j, h, nblk=nblk, q0s=q0s, exps=exps,
                                psy=psy):
                        q0 = q0s[j]
                        va = (vaA if h == 0 else vaB)[:,
                                                       j * 66:j * 66 + 128]
                        nc.tensor.matmul(
                            psy[h][:, q0:TOK], va,
                            exps.pop((j, h))[:, q0:TOK],
                            start=(j == 0), stop=(j == nblk - 1))

                    LA = 3  # QK lookahead (bounded by mm pool depth)
                    for jj in range(min(LA, nblk)):
                        emit_qk(jj, 0)
                        emit_qk(jj, 1)
                    for j in range(nblk):
                        if j + LA < nblk:
                            emit_qk(j + LA, 0)
                            emit_qk(j + LA, 1)
                        emit_pv(j, 0)
                        emit_pv(j, 1)
                        pull(2)

                    # ---- normalize (v1-proven chain), emission staged so
                    # the in-order DVE/gpsimd waits overlap pulled filler ----
                    tsl = slice(t4 * TOK, (t4 + 1) * TOK)
                    zrows, z0s, recs, sbs = [], [], [], []
                    for h in (0, 1):
                        zrow = small.tile([65, TOK], F32, tag="zrow",
                                          bufs=4, name="zrow")
                        nc.vector.tensor_copy(zrow[64:65, :],
                                              psy[h][64:65, :])
                        zrows.append(zrow)
                    for h in (0, 1):
                        z0 = small.tile([1, TOK], F32, tag="z0", bufs=4,
                                        name="z0")
                        nc.sync.dma_start(z0[:], zrows[h][64:65, :])
                        z0s.append(z0)
                    pull(2)
                    for h in (0, 1):
                        recipf = small.tile([1, TOK], F32, tag="recipf",
                                            bufs=4, name="recipf")
                        nc.vector.reciprocal_approx_fast(
                            out=recipf[:], in_=z0s[h][:])
                        recs.append(recipf)
                    for h in (0, 1):
                        sb_b = small.tile([64, TOK], F32, tag="sbb",
                                          bufs=4, name="sbb")
                        nc.gpsimd.partition_broadcast(sb_b[:], recs[h][:])
                        sbs.append(sb_b)
                    pull(3)
                    nc.vector.tensor_mul(yt[0:64, tsl], psy[0][0:64, :],
                                         sbs[0][:])
                    stage = small.tile([64, TOK], F16, tag="stage", bufs=4,
                                       name="stage")
                    nc.vector.tensor_mul(stage[:], psy[1][0:64, :],
                                         sbs[1][:])
                    nc.sync.dma_start(yt[64:128, tsl], stage[:])
                    # proj for this chunk: last batch feeds itself (no next
                    # batch exists); earlier batches feed the next attention
                    if b == B - 1:
                        fill.add_front(gen_proj_chunk(b, st, t4))
                    else:
                        st["projq"].append(gen_proj_chunk(b, st, t4))
                    if DBG:
                        ci = b * NT4 + t4
                        nc.sync.dma_start(zrd[ci * 65:(ci + 1) * 65, :],
                                          zr[:])
                        nc.sync.dma_start(r16d[ci * 65:(ci + 1) * 65, :],
                                          r16[:])
                        nc.sync.dma_start(ysd[ci * 128:(ci + 1) * 128, :],
                                          ys[:])
                # drain whatever filler remains before the next batch
                fill.drain()

            # ---- schedule: prologue b=0, then attn{b} with interleave ----
            import collections

            class Fill:
                def __init__(self):
                    self.q = collections.deque()
                    self.nofill = os.environ.get("BASSNOFILL", "0") == "1"

                def add(self, gen):
                    self.q.append(iter(gen))

                def add_front(self, gen):
                    self.q.appendleft(iter(gen))

                def pull(self, n):
                    if self.nofill:
                        return
                    while n > 0 and self.q:
                        try:
                            u = next(self.q[0])
                        except StopIteration:
                            self.q.popleft()
                            continue
                        u()
                        n -= 1

                def drain(self):
                    while self.q:
                        try:
                            u = next(self.q[0])
                        except StopIteration:
                            self.q.popleft()
                            continue
                        u()

            sts = {}
            sts[0] = alloc_batch(0)
            with nc.named_scope("prep0"):
                for u in itertools.chain(gen_qkv(0, sts[0]),
                                         gen_vtr(0, sts[0]),
                                         gen_kps(0, sts[0])):
                    u()
            for b in range(B):
                fill = Fill()
                if b > 0:
                    for g in sts[b - 1]["projq"]:
                        fill.add(g)
                if b + 1 < B:
                    sts[b + 1] = alloc_batch(b + 1)
                    fill.add(gen_qkv(b + 1, sts[b + 1]))
                    fill.add(gen_vtr(b + 1, sts[b + 1]))
                    fill.add(gen_kps(b + 1, sts[b + 1]))
                with nc.named_scope(f"attn{b}"):
                    emit_attn(b, sts[b], fill)
    nc.compile()
    return nc


def _prep_inputs(x, W_attn, b_attn, A_attn, B_attn, W_proj, b_proj, A_proj,
                 B_proj):
    xT = np.ascontiguousarray(x.reshape(BT, C).T)
    # Fold LoRA into the dense weights (exact same function, fp32 math).
    W_attn_eff = W_attn + SCALING * (B_attn.astype(np.float64)
                                     @ A_attn.astype(np.float64)
                                     ).astype(np.float32)
    W_proj_eff = W_proj + SCALING * (B_proj.astype(np.float64)
                                     @ A_proj.astype(np.float64)
                                     ).astype(np.float32)
    tri = np.triu(np.ones((128, 128), np.float32))
    eye = np.eye(128, dtype=np.float32)
    in_maps = []
    for c in range(NCORES):
        rows = np.r_[128 * c:128 * c + 128,
                     C + 128 * c:C + 128 * c + 128,
                     2 * C + 128 * c:2 * C + 128 * c + 128]
        W_sl = W_attn_eff[rows]                              # [384, C]
        WqT = np.ascontiguousarray(W_sl.T).reshape(NCIN, 128, 384)
        b_sl = np.ascontiguousarray(b_attn[rows].reshape(3, 128).T)
        ysl = slice(128 * c, 128 * c + 128)
        WpT = np.ascontiguousarray(W_proj_eff[:, ysl].T)     # [128, C]
        h = np.float16
        in_maps.append({
            "xT": xT.astype(h), "Wq": WqT.astype(h), "bq": b_sl,
            "Wp": WpT.astype(h), "tri": tri.astype(h),
            "eye": eye.astype(h),
            "onesb": np.ones((128, 16), h),
            "zerosD": np.zeros((128, 1024), h),
        })
    return in_maps


def _install_ntff_shim():
    """Provide antenv.axon_hooks (missing on this image) via ctypes against
    the axon .so, mirroring trn_agent_boot.trn_boot._ntff_profile_via_ctypes."""
    import types
    import ctypes
    import contextlib
    try:
        from antenv.axon_hooks import get_axon_ntff_profile_hook  # noqa: F401
        return
    except ImportError:
        pass
    so_path = "/opt/axon/libaxon_pjrt.so"
    try:
        lib = ctypes.CDLL(so_path)
    except OSError:
        return
    if not hasattr(lib, "axon_start_nrt_profile"):
        return
    lib.axon_start_nrt_profile.argtypes = [ctypes.POINTER(ctypes.c_int64),
                                           ctypes.c_size_t]
    lib.axon_start_nrt_profile.restype = ctypes.c_int64
    lib.axon_stop_nrt_profile.argtypes = [ctypes.c_char_p]
    lib.axon_stop_nrt_profile.restype = ctypes.c_int64

    @contextlib.contextmanager
    def _hook(output_dir, device_ids):
        import jax
        jax.devices()
        if device_ids:
            ids = (ctypes.c_int64 * len(device_ids))(*device_ids)
            rc = lib.axon_start_nrt_profile(ids, len(device_ids))
        else:
            rc = lib.axon_start_nrt_profile(None, 0)
        if rc != 0:
            raise RuntimeError(f"axon_start_nrt_profile rc={rc}")
        try:
            yield
        finally:
            n = lib.axon_stop_nrt_profile(str(output_dir).encode())
            if n < 0:
                raise RuntimeError(f"axon_stop_nrt_profile rc={n}")

    import antenv
    mod = types.ModuleType("antenv.axon_hooks")
    mod.get_axon_ntff_profile_hook = lambda: _hook
    mod.set_axon_ntff_profile_hook = lambda h: None
    sys.modules["antenv.axon_hooks"] = mod
    antenv.axon_hooks = mod


def run(inputs, trace=False, trace_cores=None):
    """Run the kernel. Returns (output, BassKernelResults)."""
    if "nc" not in _cache:
        _cache["nc"] = _build()
    nc = _cache["nc"]
    inputs = {k: np.asarray(v, dtype=np.float32) for k, v in inputs.items()}
    in_maps = _prep_inputs(**inputs)
    if trace:
        _install_ntff_shim()
    res = run_bass_kernel_spmd(nc, in_maps, core_ids=list(range(NCORES)),
                               trace=trace, trace_cores=trace_cores)
    outT = np.zeros((C, BT), np.float32)
    for r in res.results:
        outT += r["outT"].astype(np.float32)
    out = outT.T + inputs["b_proj"][None, :]
    return out.astype(np.float32).reshape(B, T, C), res


def kernel(**inputs):
    out, _ = run(inputs, trace=False)
    return out


# revision 12
# speedup vs baseline: 1.1971x; 1.1971x over previous
"""Trainium2 Bass kernel for causal multi-head attention with LoRA (QKV + proj).

Problem (hardcoded): B=4, T=2048, C=1024, NH=16, HD=64, RANK=56, alpha=8.

Sharding: tensor-parallel across heads — each of the 8 cores owns 2 heads
(128 qkv dims per projection) and processes all 4 batches. The output
projection is row-parallel (each core contracts over its own 128 y dims);
partial outputs are summed on the host.

v3 design notes:
- LoRA factors are folded into the dense weights on the host
  (W_eff = W + scaling*B@A — mathematically the same function).
- All matmuls fp16 (PSUM accumulation fp32); fp8 DoubleRow measured
  SLOWER than fp16 on this HW, so fp16 everywhere.
- ACT (scalar engine) costs ~1.4ns/col regardless of partition count or
  dtype, so ACT runs ONLY the softmax exp (~195us/core); every other
  PSUM->SBUF copy runs on DVE. Exp is the attention-phase bottleneck
  (> attention PE work), so attention is interleaved with other-phase PE
  work: during attn{b} the emission stream pulls "filler" units that run
  proj{b-1} and the QKV projection/V-transpose/K-pad prep of batch b+1
  on the otherwise-idle PE slots.
- Softmax normalize chain per (chunk, head) avoids GPSIMD/DMA partition
  moves entirely: DVE copies the sum row from PSUM, reciprocal_approx_fast,
  cast to fp16, then a K=1 matmul against a ones row broadcasts the
  per-column reciprocal across 64 partitions into PSUM; DVE multiplies
  psy (copied to SBUF fp16) by the broadcast. Head 1's PV stationary (vaB)
  stores V at block offset 64 (block stride 130, ones at offset 0..1) so
  its y lands on PSUM partitions 64..127 and its sums on partitions 0..1,
  keeping every engine op partition-base-aligned (engines cannot shift
  partitions; only DMA can, and DMA shifts cost ~600ns latency).
- PSUM budget (8 banks): mm=3 (QK scores + transposes + proj), acc=2
  (QKV accumulators), accy=2 (PV accumulators), bc=1 (recip broadcast).
- All big stationary operands are padded to full 128x128 tiles (zero-pad
  K, junk-tolerant PV slices) to stay on the PE fast-weight-load path.
"""
import os
import sys
import itertools
import numpy as np

if "/opt/trn_rl_repo" not in sys.path:
    sys.path.insert(0, "/opt/trn_rl_repo")

import concourse.bass as bass  # noqa: E402
from concourse import bacc  # noqa: E402
import concourse.mybir as mybir  # noqa: E402
import concourse.tile as tile  # noqa: E402
from concourse.bass_utils import run_bass_kernel_spmd  # noqa: E402

B, T, C = 4, 2048, 1024
NH, HD, RANK = 16, 64, 56
SCALING = 8.0 / 56.0
NCORES = 8
BT = B * T            # 8192
TOK = 512             # token chunk (matmul moving dim)
NT4 = T // TOK        # 4 token chunks per batch
NCIN = C // 128       # 8 input-feature chunks
NCO = C // 128        # 8 output-feature chunks (proj)
F32 = mybir.dt.float32
F16 = mybir.dt.float16
EXPF = mybir.ActivationFunctionType.Exp
COPYF = mybir.ActivationFunctionType.Identity

_cache = {}


def _build():
    nc = bacc.Bacc("TRN2", target_bir_lowering=False, debug=False,
                   num_devices=NCORES)
    xT = nc.dram_tensor("xT", [C, BT], F16, kind="ExternalInput")
    Wq = nc.dram_tensor("Wq", [NCIN, 128, 384], F16, kind="ExternalInput")
    bq = nc.dram_tensor("bq", [128, 3], F32, kind="ExternalInput")
    Wp = nc.dram_tensor("Wp", [128, C], F16, kind="ExternalInput")
    tri = nc.dram_tensor("tri", [128, 128], F16, kind="ExternalInput")
    eye = nc.dram_tensor("eye", [128, 128], F16, kind="ExternalInput")
    onesb = nc.dram_tensor("onesb", [128, 16], F16, kind="ExternalInput")
    zerosD = nc.dram_tensor("zerosD", [128, 1024], F16, kind="ExternalInput")
    outT = nc.dram_tensor("outT", [C, BT], F16, kind="ExternalOutput")
    DBG = os.environ.get("BASSDBG", "0") == "1"
    if DBG:
        zrd = nc.dram_tensor("zrd", [16 * 65, TOK], F32,
                             kind="ExternalOutput")
        r16d = nc.dram_tensor("r16d", [16 * 65, TOK], F16,
                              kind="ExternalOutput")
        ysd = nc.dram_tensor("ysd", [16 * 128, TOK], F16,
                             kind="ExternalOutput")

    with tile.TileContext(nc) as tc:
        with (
            tc.tile_pool(name="consts", bufs=1) as consts,
            tc.tile_pool(name="qkv", bufs=2) as qkvp,
            tc.tile_pool(name="persist", bufs=1) as persist,
            tc.tile_pool(name="ytp", bufs=2) as ytp,
            tc.tile_pool(name="xtp", bufs=16) as xtp,
            tc.tile_pool(name="expp", bufs=12) as expp,
            tc.tile_pool(name="small", bufs=2) as small,
            tc.tile_pool(name="ps", bufs=1, space="PSUM") as ps,
        ):
            wq_sb = consts.tile([128, NCIN, 384], F16)
            nc.sync.dma_start(wq_sb[:], Wq[:].rearrange("c p f -> p c f"))
            bias_sb = consts.tile([128, 3], F32)
            nc.sync.dma_start(bias_sb[:], bq[:])
            wp_sb = consts.tile([128, C], F16)
            nc.sync.dma_start(wp_sb[:], Wp[:])
            tri_sb = consts.tile([128, 128], F16)
            nc.sync.dma_start(tri_sb[:], tri[:])
            eye_sb = consts.tile([128, 128], F16)
            nc.sync.dma_start(eye_sb[:], eye[:])
            zeros_sb = consts.tile([128, TOK], F16)
            nc.sync.dma_start(zeros_sb[:], zerosD[:, 0:TOK])

            # persistent padded-K tiles and V-aug tiles, two sets
            # alternated by batch parity; zero/ones regions written once
            # through these same tile objects (so every later read is
            # dependency-tracked).
            kpset, vaset = {}, {}
            for par in range(2):
                for j in range(16):
                    for hh in range(2):
                        kpset[(par, j, hh)] = persist.tile(
                            [128, 128], F16, name=f"kp{par}_{j}_{hh}")
                for hh in range(2):
                    vaset[(par, hh)] = persist.tile(
                        [128, 16 * 66 + 64], F16, name=f"va{par}_{hh}")
            for par in range(2):
                for j in range(16):
                    nc.vector.tensor_copy(kpset[(par, j, 0)][64:128, :],
                                          zeros_sb[64:128, 0:128])
                    nc.vector.tensor_copy(kpset[(par, j, 1)][0:64, :],
                                          zeros_sb[0:64, 0:128])
                for hh in range(2):
                    va_ = vaset[(par, hh)]
                    nc.vector.tensor_copy(va_[:, 1056:1120],
                                          zeros_sb[:, 0:64])
                    vav_ = va_[:, 0:1056].rearrange("p (j c) -> p j c",
                                                    c=66)
                    for col in (64, 65):
                        nc.sync.dma_start(vav_[:, :, col:col + 1],
                                          onesb[:].unsqueeze(-1))

            # ---- per-batch persistent tiles ----
            def alloc_batch(b):
                st = {}
                st["qT"] = qkvp.tile([128, T], F16, tag="qT", name="qT")
                st["kT"] = qkvp.tile([128, T], F16, tag="kT", name="kT")
                st["vT"] = qkvp.tile([128, T], F16, tag="vT", name="vT")
                st["vaA"] = vaset[(b % 2, 0)]
                st["vaB"] = vaset[(b % 2, 1)]
                st["yt"] = ytp.tile([128, T], F16, tag="yt", name="yt")
                st["kps"] = {j: (kpset[(b % 2, j, 0)], kpset[(b % 2, j, 1)])
                             for j in range(16)}
                st["projq"] = []
                return st

            def gen_qkv(b, st):
                """QKV projection units for batch b (7 units per t4)."""
                qkvd = (st["qT"], st["kT"], st["vT"])
                for t4 in range(NT4):
                    hold = {}

                    def u_load(t4=t4, hold=hold):
                        gcol = b * T + t4 * TOK
                        xts = []
                        for cc in range(NCIN):
                            xt = xtp.tile([128, TOK], F16, tag="xt")
                            nc.sync.dma_start(
                                xt[:],
                                xT[cc * 128:(cc + 1) * 128,
                                   gcol:gcol + TOK])
                            xts.append(xt)
                        hold["x"] = xts
                    yield u_load
                    for ch in range(3):
                        def u_mm1(ch=ch, t4=t4, hold=hold):
                            ps_q = ps.tile([128, TOK], F32, tag="acc",
                                           bufs=2, name="psq")
                            hold[("p", ch)] = ps_q
                            for cc in range(4):
                                nc.tensor.matmul(
                                    ps_q[:],
                                    wq_sb[:, cc, ch * 128:(ch + 1) * 128],
                                    hold["x"][cc][:], start=(cc == 0),
                                    stop=False)

                        def u_mm2(ch=ch, t4=t4, hold=hold):
                            ps_q = hold.pop(("p", ch))
                            for cc in range(4, 8):
                                nc.tensor.matmul(
                                    ps_q[:],
                                    wq_sb[:, cc, ch * 128:(ch + 1) * 128],
                                    hold["x"][cc][:], start=False,
                                    stop=(cc == NCIN - 1))
                            nc.scalar.activation(
                                out=qkvd[ch][:, t4 * TOK:(t4 + 1) * TOK],
                                in_=ps_q[:], func=COPYF,
                                bias=bias_sb[:, ch:ch + 1], scale=1.0)
                        yield u_mm1
                        yield u_mm2

            def gen_vtr(b, st):
                """V -> token-major into vaA (h0) and vaB (h1, offset 64)."""
                vaA, vaB, vT = st["vaA"], st["vaB"], st["vT"]
                vaAv = vaA[:, 0:1056].rearrange("p (j c) -> p j c", c=66)
                vaBv = vaB[:, 0:1056].rearrange("p (j c) -> p j c", c=66)

                for tb in range(16):
                    def u(tb=tb):
                        ps_t = ps.tile([128, 128], F16, tag="mm", bufs=3)
                        nc.tensor.transpose(
                            ps_t[:], vT[:, tb * 128:(tb + 1) * 128],
                            eye_sb[:])
                        nc.vector.tensor_copy(vaAv[:, tb, 0:64],
                                              ps_t[:, 0:64])
                        nc.vector.tensor_copy(vaBv[:, tb, 0:64],
                                              ps_t[:, 64:128])
                    yield u

            def gen_kps(b, st):
                """Zero-padded per-head K tiles (head select via zeros)."""
                kT, kps = st["kT"], st["kps"]
                for j in range(16):
                    def u(j=j):
                        kp0, kp1 = kps[j]
                        nc.vector.tensor_copy(
                            kp0[0:64, :], kT[0:64, j * 128:(j + 1) * 128])
                        nc.vector.tensor_copy(
                            kp1[64:128, :],
                            kT[64:128, j * 128:(j + 1) * 128])
                    yield u

            def gen_proj_chunk(b, st, t4):
                """Output projection units for chunk t4 of batch b."""
                yt = st["yt"]
                if True:
                    for co in range(NCO):
                        def u(t4=t4, co=co):
                            gcol = b * T + t4 * TOK
                            ps_o = ps.tile([128, TOK], F32, tag="mm",
                                           bufs=3)
                            nc.tensor.matmul(
                                ps_o[:],
                                wp_sb[:, co * 128:(co + 1) * 128],
                                yt[:, t4 * TOK:(t4 + 1) * TOK],
                                start=True, stop=True)
                            po = small.tile([128, TOK], F16, tag="po",
                                            bufs=3)
                            nc.vector.tensor_copy(po[:], ps_o[:])
                            nc.sync.dma_start(
                                outT[co * 128:(co + 1) * 128,
                                     gcol:gcol + TOK], po[:])
                        yield u

            def emit_attn(b, st, fill):
                """Attention for batch b, pulling filler units into PE gaps."""
                qT, vaA, vaB, yt, kps = (st["qT"], st["vaA"], st["vaB"],
                                         st["yt"], st["kps"])
                pull = fill.pull

                for t4 in range(NT4):
                    nblk = 4 * (t4 + 1)
                    q0s, exps = {}, {}
                    psy0 = ps.tile([128, TOK], F32, tag="accy", bufs=3)
                    psy1 = ps.tile([128, TOK], F32, tag="accy", bufs=3)
                    psy = {0: psy0, 1: psy1}

                    def emit_qk(j, h, t4=t4, q0s=q0s, exps=exps, kps=kps):
                        r = j - 4 * t4
                        q0 = 128 * r if r > 0 else 0
                        q0s[j] = q0
                        ps_s = ps.tile([128, TOK], F32, tag="mm", bufs=3)
                        nc.tensor.matmul(
                            ps_s[:, q0:TOK],
                            kps[j][h][:],
                            qT[:, t4 * TOK + q0:(t4 + 1) * TOK],
                            start=True, stop=True)
                        e = expp.tile([128, TOK], F16, tag="expS")
                        nc.scalar.activation(
                            out=e[:, q0:TOK], in_=ps_s[:, q0:TOK],
                            func=EXPF, scale=0.125)
                        if r >= 0:
                            nc.vector.tensor_mul(
                                e[:, q0:q0 + 128], e[:, q0:q0 + 128],
                                tri_sb[:])
                        exps[(j, h)] = e

                    def emit_pv(j, h, nblk=nblk, q0s=q0s, exps=exps,
                                psy=psy):
                        q0 = q0s[j]
                        va = (vaA if h == 0 else vaB)[:,
                                                       j * 66:j * 66 + 128]
                        nc.tensor.matmul(
                            psy[h][:, q0:TOK], va,
                            exps.pop((j, h))[:, q0:TOK],
                            start=(j == 0), stop=(j == nblk - 1))

                    LA = 3  # QK lookahead (bounded by mm pool depth)
                    for jj in range(min(LA, nblk)):
                        emit_qk(jj, 0)
                        emit_qk(jj, 1)
                    for j in range(nblk):
                        if j + LA < nblk:
                            emit_qk(j + LA, 0)
                            emit_qk(j + LA, 1)
                        emit_pv(j, 0)
                        emit_pv(j, 1)
                        pull(2)

                    # ---- normalize (v1-proven): DMA shift + gpsimd ----
                    tsl = slice(t4 * TOK, (t4 + 1) * TOK)
                    for h in (0, 1):
                        zrow = small.tile([65, TOK], F32, tag="zrow",
                                          bufs=4, name="zrow")
                        nc.vector.tensor_copy(zrow[64:65, :],
                                              psy[h][64:65, :])
                        z0 = small.tile([1, TOK], F32, tag="z0", bufs=4,
                                        name="z0")
                        nc.sync.dma_start(z0[:], zrow[64:65, :])
                        recipf = small.tile([1, TOK], F32, tag="recipf",
                                            bufs=4, name="recipf")
                        nc.vector.reciprocal_approx_fast(
                            out=recipf[:], in_=z0[:])
                        sb_b = small.tile([64, TOK], F32, tag="sbb",
                                          bufs=4, name="sbb")
                        nc.gpsimd.partition_broadcast(sb_b[:], recipf[:])
                        if h == 0:
                            nc.vector.tensor_mul(yt[0:64, tsl],
                                                 psy[0][0:64, :], sb_b[:])
                        else:
                            stage = small.tile([64, TOK], F16, tag="stage",
                                               bufs=4, name="stage")
                            nc.vector.tensor_mul(stage[:],
                                                 psy[1][0:64, :], sb_b[:])
                            nc.sync.dma_start(yt[64:128, tsl], stage[:])
                        pull(1)
                    # proj for this chunk: last batch feeds itself (no next
                    # batch exists); earlier batches feed the next attention
                    if b == B - 1:
                        fill.add_front(gen_proj_chunk(b, st, t4))
                    else:
                        st["projq"].append(gen_proj_chunk(b, st, t4))
                    if DBG:
                        ci = b * NT4 + t4
                        nc.sync.dma_start(zrd[ci * 65:(ci + 1) * 65, :],
                                          zr[:])
                        nc.sync.dma_start(r16d[ci * 65:(ci + 1) * 65, :],
                                          r16[:])
                        nc.sync.dma_start(ysd[ci * 128:(ci + 1) * 128, :],
                                          ys[:])
                # drain whatever filler remains before the next batch
                fill.drain()

            # ---- schedule: prologue b=0, then attn{b} with interleave ----
            import collections

            class Fill:
                def __init__(self):
                    self.q = collections.deque()
                    self.nofill = os.environ.get("BASSNOFILL", "0") == "1"

                def add(self, gen):
                    self.q.append(iter(gen))

                def add_front(self, gen):
                    self.q.appendleft(iter(gen))

                def pull(self, n):
                    if self.nofill:
                        return
                    while n > 0 and self.q:
                        try:
                            u = next(self.q[0])
                        except StopIteration:
                            self.q.popleft()
                            continue
                        u()
                        n -= 1

                def drain(self):
                    while self.q:
                        try:
                            u = next(self.q[0])
                        except StopIteration:
                            self.q.popleft()
                            continue
                        u()

            sts = {}
            sts[0] = alloc_batch(0)
            with nc.named_scope("prep0"):
                for u in itertools.chain(gen_qkv(0, sts[0]),
                                         gen_vtr(0, sts[0]),
                                         gen_kps(0, sts[0])):
                    u()
            for b in range(B):
                fill = Fill()
                if b > 0:
                    for g in sts[b - 1]["projq"]:
                        fill.add(g)
                if b + 1 < B:
                    sts[b + 1] = alloc_batch(b + 1)
                    fill.add(gen_qkv(b + 1, sts[b + 1]))
                    fill.add(gen_vtr(b + 1, sts[b + 1]))
                    fill.add(gen_kps(b + 1, sts[b + 1]))
                with nc.named_scope(f"attn{b}"):
                    emit_attn(b, sts[b], fill)
    nc.compile()
    return nc


def _prep_inputs(x, W_attn, b_attn, A_attn, B_attn, W_proj, b_proj, A_proj,
                 B_proj):
    xT = np.ascontiguousarray(x.reshape(BT, C).T)
    # Fold LoRA into the dense weights (exact same function, fp32 math).
    W_attn_eff = W_attn + SCALING * (B_attn.astype(np.float64)
                                     @ A_attn.astype(np.float64)
                                     ).astype(np.float32)
    W_proj_eff = W_proj + SCALING * (B_proj.astype(np.float64)
                                     @ A_proj.astype(np.float64)
                                     ).astype(np.float32)
    tri = np.triu(np.ones((128, 128), np.float32))
    eye = np.eye(128, dtype=np.float32)
    in_maps = []
    for c in range(NCORES):
        rows = np.r_[128 * c:128 * c + 128,
                     C + 128 * c:C + 128 * c + 128,
                     2 * C + 128 * c:2 * C + 128 * c + 128]
        W_sl = W_attn_eff[rows]                              # [384, C]
        WqT = np.ascontiguousarray(W_sl.T).reshape(NCIN, 128, 384)
        b_sl = np.ascontiguousarray(b_attn[rows].reshape(3, 128).T)
        ysl = slice(128 * c, 128 * c + 128)
        WpT = np.ascontiguousarray(W_proj_eff[:, ysl].T)     # [128, C]
        h = np.float16
        in_maps.append({
            "xT": xT.astype(h), "Wq": WqT.astype(h), "bq": b_sl,
            "Wp": WpT.astype(h), "tri": tri.astype(h),
            "eye": eye.astype(h),
            "onesb": np.ones((128, 16), h),
            "zerosD": np.zeros((128, 1024), h),
        })
    return in_maps


def _install_ntff_shim():
    """Provide antenv.axon_hooks (missing on this image) via ctypes against
    the axon .so, mirroring trn_agent_boot.trn_boot._ntff_profile_via_ctypes."""
    import types
    import ctypes
    import contextlib
    try:
        from antenv.axon_hooks import get_axon_ntff_profile_hook  # noqa: F401
        return
    except ImportError:
        pass
    so_path = "/opt/axon/libaxon_pjrt.so"
    try:
        lib = ctypes.CDLL(so_path)
    except OSError:
        return
    if not hasattr(lib, "axon_start_nrt_profile"):
        return
    lib.axon_start_nrt_profile.argtypes = [ctypes.POINTER(ctypes.c_int64),
                                           ctypes.c_size_t]
    lib.axon_start_nrt_profile.restype = ctypes.c_int64
    lib.axon_stop_nrt_profile.argtypes = [ctypes.c_char_p]
    lib.axon_stop_nrt_profile.restype = ctypes.c_int64

    @contextlib.contextmanager
    def _hook(output_dir, device_ids):
        import jax
        jax.devices()
        if device_ids:
            ids = (ctypes.c_int64 * len(device_ids))(*device_ids)
            rc = lib.axon_start_nrt_profile(ids, len(device_ids))
        else:
            rc = lib.axon_start_nrt_profile(None, 0)
        if rc != 0:
            raise RuntimeError(f"axon_start_nrt_profile rc={rc}")
        try:
            yield
        finally:
            n = lib.axon_stop_nrt_profile(str(output_dir).encode())
            if n < 0:
                raise RuntimeError(f"axon_stop_nrt_profile rc={n}")

    import antenv
    mod = types.ModuleType("antenv.axon_hooks")
    mod.get_axon_ntff_profile_hook = lambda: _hook
    mod.set_axon_ntff_profile_hook = lambda h: None
    sys.modules["antenv.axon_hooks"] = mod
    antenv.axon_hooks = mod


def run(inputs, trace=False, trace_cores=None):
    """Run the kernel. Returns (output, BassKernelResults)."""
    if "nc" not in _cache:
        _cache["nc"] = _build()
    nc = _cache["nc"]
    inputs = {k: np.asarray(v, dtype=np.float32) for k, v in inputs.items()}
    in_maps = _prep_inputs(**inputs)
    if trace:
        _install_ntff_shim()
    res = run_bass_kernel_spmd(nc, in_maps, core_ids=list(range(NCORES)),
                               trace=trace, trace_cores=trace_cores)
    outT = np.zeros((C, BT), np.float32)
    for r in res.results:
        outT += r["outT"].astype(np.float32)
    out = outT.T + inputs["b_proj"][None, :]
    return out.astype(np.float32).reshape(B, T, C), res


def kernel(**inputs):
    out, _ = run(inputs, trace=False)
    return out


# revision 13
# speedup vs baseline: 1.2158x; 1.0157x over previous
"""Trainium2 Bass kernel for causal multi-head attention with LoRA (QKV + proj).

Problem (hardcoded): B=4, T=2048, C=1024, NH=16, HD=64, RANK=56, alpha=8.

Sharding: tensor-parallel across heads — each of the 8 cores owns 2 heads
(128 qkv dims per projection) and processes all 4 batches. The output
projection is row-parallel (each core contracts over its own 128 y dims);
partial outputs are summed on the host.

Design (measured on this HW stack; see mb.py / mb2.py microbenches):
- LoRA factors are folded into the dense weights on the host
  (W_eff = W + scaling*B@A — mathematically the same function).
- All matmuls fp16, PSUM accumulation fp32. fp8 DoubleRow measured SLOWER
  than fp16 here, so fp16 everywhere.
- ACT (scalar engine) costs ~1.4ns/col regardless of partitions/dtype, so
  ACT runs only the softmax exp (~195us/core) plus the QKV bias-copies;
  exp throughput exceeds attention PE work, so attention is interleaved
  with other-phase PE work: the emission stream for attn{b} pulls "filler"
  units that run proj{b-1} and batch b+1's QKV projection / V-transpose /
  K-pad prep in the PE gaps (Fill queue; pull(2) per attention block,
  pull(1) after each normalize head — pacing sized so supply lasts through
  the longest chunks). The last batch feeds its own proj chunks back into
  the filler as each yt chunk completes.
- Softmax: scoresT [tk, tq] -> exp on ACT (scale=1/8 folded in); causal
  masking via block-sliced matmul ranges + one [128,128] triangular mask
  multiply per diagonal block; row sums via ones columns appended to the
  PV stationary operand. Normalize per (chunk, head): DVE copies the sum
  row from PSUM, SBUF->SBUF DMA shifts it to partition 0,
  reciprocal_approx_fast, gpsimd partition_broadcast, DVE multiply.
  (A K=1 cross-partition-base broadcast matmul is numerically correct in
  CoreSim but corrupts concurrent PSUM banks on real HW — do not.)
- K-pad and V-aug stationary tiles are PERSISTENT (two sets alternated by
  batch parity), zero/ones regions written once through the same tile
  objects so every read is dependency-tracked (ring-buffer reuse of
  zero-padding across tile lifetimes raced on HW).
- PSUM budget (8 banks): mm=3 (QK scores + transposes + proj), acc=2
  (QKV accumulators), accy=3 (PV accumulators).
- All big stationary operands are padded to full 128x128 tiles to stay on
  the PE fast-weight-load path.
History: v1 469us -> LoRA fold 445us -> interleave+scheduling ~400us
(trace-measured; rel err vs fp32 reference ~5.9e-4).
"""
import os
import sys
import itertools
import numpy as np

if "/opt/trn_rl_repo" not in sys.path:
    sys.path.insert(0, "/opt/trn_rl_repo")

import concourse.bass as bass  # noqa: E402
from concourse import bacc  # noqa: E402
import concourse.mybir as mybir  # noqa: E402
import concourse.tile as tile  # noqa: E402
from concourse.bass_utils import run_bass_kernel_spmd  # noqa: E402

B, T, C = 4, 2048, 1024
NH, HD, RANK = 16, 64, 56
SCALING = 8.0 / 56.0
NCORES = 8
BT = B * T            # 8192
TOK = 512             # token chunk (matmul moving dim)
NT4 = T // TOK        # 4 token chunks per batch
NCIN = C // 128       # 8 input-feature chunks
NCO = C // 128        # 8 output-feature chunks (proj)
F32 = mybir.dt.float32
F16 = mybir.dt.float16
EXPF = mybir.ActivationFunctionType.Exp
COPYF = mybir.ActivationFunctionType.Identity

_cache = {}


def _build():
    nc = bacc.Bacc("TRN2", target_bir_lowering=False, debug=False,
                   num_devices=NCORES)
    xT = nc.dram_tensor("xT", [C, BT], F16, kind="ExternalInput")
    Wq = nc.dram_tensor("Wq", [NCIN, 128, 384], F16, kind="ExternalInput")
    bq = nc.dram_tensor("bq", [128, 3], F32, kind="ExternalInput")
    Wp = nc.dram_tensor("Wp", [128, C], F16, kind="ExternalInput")
    tri = nc.dram_tensor("tri", [128, 128], F16, kind="ExternalInput")
    eye = nc.dram_tensor("eye", [128, 128], F16, kind="ExternalInput")
    onesb = nc.dram_tensor("onesb", [128, 16], F16, kind="ExternalInput")
    zerosD = nc.dram_tensor("zerosD", [128, 1024], F16, kind="ExternalInput")
    outT = nc.dram_tensor("outT", [C, BT], F16, kind="ExternalOutput")
    DBG = os.environ.get("BASSDBG", "0") == "1"
    if DBG:
        zrd = nc.dram_tensor("zrd", [16 * 65, TOK], F32,
                             kind="ExternalOutput")
        r16d = nc.dram_tensor("r16d", [16 * 65, TOK], F16,
                              kind="ExternalOutput")
        ysd = nc.dram_tensor("ysd", [16 * 128, TOK], F16,
                             kind="ExternalOutput")

    with tile.TileContext(nc) as tc:
        with (
            tc.tile_pool(name="consts", bufs=1) as consts,
            tc.tile_pool(name="qkv", bufs=2) as qkvp,
            tc.tile_pool(name="persist", bufs=1) as persist,
            tc.tile_pool(name="ytp", bufs=2) as ytp,
            tc.tile_pool(name="xtp", bufs=16) as xtp,
            tc.tile_pool(name="expp", bufs=12) as expp,
            tc.tile_pool(name="small", bufs=2) as small,
            tc.tile_pool(name="ps", bufs=1, space="PSUM") as ps,
        ):
            wq_sb = consts.tile([128, NCIN, 384], F16)
            nc.sync.dma_start(wq_sb[:], Wq[:].rearrange("c p f -> p c f"))
            bias_sb = consts.tile([128, 3], F32)
            nc.sync.dma_start(bias_sb[:], bq[:])
            wp_sb = consts.tile([128, C], F16)
            nc.sync.dma_start(wp_sb[:], Wp[:])
            tri_sb = consts.tile([128, 128], F16)
            nc.sync.dma_start(tri_sb[:], tri[:])
            eye_sb = consts.tile([128, 128], F16)
            nc.sync.dma_start(eye_sb[:], eye[:])
            zeros_sb = consts.tile([128, TOK], F16)
            nc.sync.dma_start(zeros_sb[:], zerosD[:, 0:TOK])

            # persistent padded-K tiles and V-aug tiles, two sets
            # alternated by batch parity; zero/ones regions written once
            # through these same tile objects (so every later read is
            # dependency-tracked).
            kpset, vaset = {}, {}
            for par in range(2):
                for j in range(16):
                    for hh in range(2):
                        kpset[(par, j, hh)] = persist.tile(
                            [128, 128], F16, name=f"kp{par}_{j}_{hh}")
                for hh in range(2):
                    vaset[(par, hh)] = persist.tile(
                        [128, 16 * 66 + 64], F16, name=f"va{par}_{hh}")
            for par in range(2):
                for j in range(16):
                    nc.vector.tensor_copy(kpset[(par, j, 0)][64:128, :],
                                          zeros_sb[64:128, 0:128])
                    nc.vector.tensor_copy(kpset[(par, j, 1)][0:64, :],
                                          zeros_sb[0:64, 0:128])
                for hh in range(2):
                    va_ = vaset[(par, hh)]
                    nc.vector.tensor_copy(va_[:, 1056:1120],
                                          zeros_sb[:, 0:64])
                    vav_ = va_[:, 0:1056].rearrange("p (j c) -> p j c",
                                                    c=66)
                    for col in (64, 65):
                        nc.sync.dma_start(vav_[:, :, col:col + 1],
                                          onesb[:].unsqueeze(-1))

            # ---- per-batch persistent tiles ----
            def alloc_batch(b):
                st = {}
                st["qT"] = qkvp.tile([128, T], F16, tag="qT", name="qT")
                st["kT"] = qkvp.tile([128, T], F16, tag="kT", name="kT")
                st["vT"] = qkvp.tile([128, T], F16, tag="vT", name="vT")
                st["vaA"] = vaset[(b % 2, 0)]
                st["vaB"] = vaset[(b % 2, 1)]
                st["yt"] = ytp.tile([128, T], F16, tag="yt", name="yt")
                st["kps"] = {j: (kpset[(b % 2, j, 0)], kpset[(b % 2, j, 1)])
                             for j in range(16)}
                st["projq"] = []
                return st

            def gen_qkv(b, st):
                """QKV projection units for batch b (7 units per t4)."""
                qkvd = (st["qT"], st["kT"], st["vT"])
                for t4 in range(NT4):
                    hold = {}

                    def u_load(t4=t4, hold=hold):
                        gcol = b * T + t4 * TOK
                        xts = []
                        for cc in range(NCIN):
                            xt = xtp.tile([128, TOK], F16, tag="xt")
                            nc.sync.dma_start(
                                xt[:],
                                xT[cc * 128:(cc + 1) * 128,
                                   gcol:gcol + TOK])
                            xts.append(xt)
                        hold["x"] = xts
                    yield u_load
                    for ch in range(3):
                        def u_mm1(ch=ch, t4=t4, hold=hold):
                            ps_q = ps.tile([128, TOK], F32, tag="acc",
                                           bufs=2, name="psq")
                            hold[("p", ch)] = ps_q
                            for cc in range(4):
                                nc.tensor.matmul(
                                    ps_q[:],
                                    wq_sb[:, cc, ch * 128:(ch + 1) * 128],
                                    hold["x"][cc][:], start=(cc == 0),
                                    stop=False)

                        def u_mm2(ch=ch, t4=t4, hold=hold):
                            ps_q = hold.pop(("p", ch))
                            for cc in range(4, 8):
                                nc.tensor.matmul(
                                    ps_q[:],
                                    wq_sb[:, cc, ch * 128:(ch + 1) * 128],
                                    hold["x"][cc][:], start=False,
                                    stop=(cc == NCIN - 1))
                            nc.scalar.activation(
                                out=qkvd[ch][:, t4 * TOK:(t4 + 1) * TOK],
                                in_=ps_q[:], func=COPYF,
                                bias=bias_sb[:, ch:ch + 1], scale=1.0)
                        yield u_mm1
                        yield u_mm2

            def gen_vtr(b, st):
                """V -> token-major into vaA (h0) and vaB (h1, offset 64)."""
                vaA, vaB, vT = st["vaA"], st["vaB"], st["vT"]
                vaAv = vaA[:, 0:1056].rearrange("p (j c) -> p j c", c=66)
                vaBv = vaB[:, 0:1056].rearrange("p (j c) -> p j c", c=66)

                for tb in range(16):
                    def u(tb=tb):
                        ps_t = ps.tile([128, 128], F16, tag="mm", bufs=3)
                        nc.tensor.transpose(
                            ps_t[:], vT[:, tb * 128:(tb + 1) * 128],
                            eye_sb[:])
                        nc.vector.tensor_copy(vaAv[:, tb, 0:64],
                                              ps_t[:, 0:64])
                        nc.vector.tensor_copy(vaBv[:, tb, 0:64],
                                              ps_t[:, 64:128])
                    yield u

            def gen_kps(b, st):
                """Zero-padded per-head K tiles (head select via zeros)."""
                kT, kps = st["kT"], st["kps"]
                for j in range(16):
                    def u(j=j):
                        kp0, kp1 = kps[j]
                        nc.vector.tensor_copy(
                            kp0[0:64, :], kT[0:64, j * 128:(j + 1) * 128])
                        nc.vector.tensor_copy(
                            kp1[64:128, :],
                            kT[64:128, j * 128:(j + 1) * 128])
                    yield u

            def gen_proj_chunk(b, st, t4):
                """Output projection units for chunk t4 of batch b."""
                yt = st["yt"]
                if True:
                    for co in range(NCO):
                        def u(t4=t4, co=co):
                            gcol = b * T + t4 * TOK
                            ps_o = ps.tile([128, TOK], F32, tag="mm",
                                           bufs=3)
                            nc.tensor.matmul(
                                ps_o[:],
                                wp_sb[:, co * 128:(co + 1) * 128],
                                yt[:, t4 * TOK:(t4 + 1) * TOK],
                                start=True, stop=True)
                            po = small.tile([128, TOK], F16, tag="po",
                                            bufs=3)
                            nc.vector.tensor_copy(po[:], ps_o[:])
                            nc.sync.dma_start(
                                outT[co * 128:(co + 1) * 128,
                                     gcol:gcol + TOK], po[:])
                        yield u

            def emit_attn(b, st, fill):
                """Attention for batch b, pulling filler units into PE gaps."""
                qT, vaA, vaB, yt, kps = (st["qT"], st["vaA"], st["vaB"],
                                         st["yt"], st["kps"])
                pull = fill.pull

                for t4 in range(NT4):
                    nblk = 4 * (t4 + 1)
                    q0s, exps = {}, {}
                    psy0 = ps.tile([128, TOK], F32, tag="accy", bufs=3)
                    psy1 = ps.tile([128, TOK], F32, tag="accy", bufs=3)
                    psy = {0: psy0, 1: psy1}

                    def emit_qk(j, h, t4=t4, q0s=q0s, exps=exps, kps=kps):
                        r = j - 4 * t4
                        q0 = 128 * r if r > 0 else 0
                        q0s[j] = q0
                        ps_s = ps.tile([128, TOK], F32, tag="mm", bufs=3)
                        nc.tensor.matmul(
                            ps_s[:, q0:TOK],
                            kps[j][h][:],
                            qT[:, t4 * TOK + q0:(t4 + 1) * TOK],
                            start=True, stop=True)
                        e = expp.tile([128, TOK], F16, tag="expS")
                        nc.scalar.activation(
                            out=e[:, q0:TOK], in_=ps_s[:, q0:TOK],
                            func=EXPF, scale=0.125)
                        if r >= 0:
                            nc.vector.tensor_mul(
                                e[:, q0:q0 + 128], e[:, q0:q0 + 128],
                                tri_sb[:])
                        exps[(j, h)] = e

                    def emit_pv(j, h, nblk=nblk, q0s=q0s, exps=exps,
                                psy=psy):
                        q0 = q0s[j]
                        va = (vaA if h == 0 else vaB)[:,
                                                       j * 66:j * 66 + 128]
                        nc.tensor.matmul(
                            psy[h][:, q0:TOK], va,
                            exps.pop((j, h))[:, q0:TOK],
                            start=(j == 0), stop=(j == nblk - 1))

                    LA = 3  # QK lookahead (bounded by mm pool depth)
                    for jj in range(min(LA, nblk)):
                        emit_qk(jj, 0)
                        emit_qk(jj, 1)
                    for j in range(nblk):
                        if j + LA < nblk:
                            emit_qk(j + LA, 0)
                            emit_qk(j + LA, 1)
                        emit_pv(j, 0)
                        emit_pv(j, 1)
                        pull(2)

                    # ---- normalize (v1-proven): DMA shift + gpsimd ----
                    tsl = slice(t4 * TOK, (t4 + 1) * TOK)
                    for h in (0, 1):
                        zrow = small.tile([65, TOK], F32, tag="zrow",
                                          bufs=4, name="zrow")
                        nc.vector.tensor_copy(zrow[64:65, :],
                                              psy[h][64:65, :])
                        z0 = small.tile([1, TOK], F32, tag="z0", bufs=4,
                                        name="z0")
                        nc.sync.dma_start(z0[:], zrow[64:65, :])
                        recipf = small.tile([1, TOK], F32, tag="recipf",
                                            bufs=4, name="recipf")
                        nc.vector.reciprocal_approx_fast(
                            out=recipf[:], in_=z0[:])
                        sb_b = small.tile([64, TOK], F32, tag="sbb",
                                          bufs=4, name="sbb")
                        nc.gpsimd.partition_broadcast(sb_b[:], recipf[:])
                        if h == 0:
                            nc.vector.tensor_mul(yt[0:64, tsl],
                                                 psy[0][0:64, :], sb_b[:])
                        else:
                            stage = small.tile([64, TOK], F16, tag="stage",
                                               bufs=4, name="stage")
                            nc.vector.tensor_mul(stage[:],
                                                 psy[1][0:64, :], sb_b[:])
                            nc.sync.dma_start(yt[64:128, tsl], stage[:])
                        pull(1)
                    # proj for this chunk: last batch feeds itself (no next
                    # batch exists); earlier batches feed the next attention
                    if b == B - 1:
                        fill.add_front(gen_proj_chunk(b, st, t4))
                    else:
                        st["projq"].append(gen_proj_chunk(b, st, t4))
                    if DBG:
                        ci = b * NT4 + t4
                        nc.sync.dma_start(zrd[ci * 65:(ci + 1) * 65, :],
                                          zr[:])
                        nc.sync.dma_start(r16d[ci * 65:(ci + 1) * 65, :],
                                          r16[:])
                        nc.sync.dma_start(ysd[ci * 128:(ci + 1) * 128, :],
                                          ys[:])
                # drain whatever filler remains before the next batch
                fill.drain()

            # ---- schedule: prologue b=0, then attn{b} with interleave ----
            import collections

            class Fill:
                def __init__(self):
                    self.q = collections.deque()
                    self.nofill = os.environ.get("BASSNOFILL", "0") == "1"

                def add(self, gen):
                    self.q.append(iter(gen))

                def add_front(self, gen):
                    self.q.appendleft(iter(gen))

                def pull(self, n):
                    if self.nofill:
                        return
                    while n > 0 and self.q:
                        try:
                            u = next(self.q[0])
                        except StopIteration:
                            self.q.popleft()
                            continue
                        u()
                        n -= 1

                def drain(self):
                    while self.q:
                        try:
                            u = next(self.q[0])
                        except StopIteration:
                            self.q.popleft()
                            continue
                        u()

            sts = {}
            sts[0] = alloc_batch(0)
            with nc.named_scope("prep0"):
                for u in itertools.chain(gen_qkv(0, sts[0]),
                                         gen_vtr(0, sts[0]),
                                         gen_kps(0, sts[0])):
                    u()
            for b in range(B):
                fill = Fill()
                if b > 0:
                    for g in sts[b - 1]["projq"]:
                        fill.add(g)
                if b + 1 < B:
                    sts[b + 1] = alloc_batch(b + 1)
                    fill.add(gen_qkv(b + 1, sts[b + 1]))
                    fill.add(gen_vtr(b + 1, sts[b + 1]))
                    fill.add(gen_kps(b + 1, sts[b + 1]))
                with nc.named_scope(f"attn{b}"):
                    emit_attn(b, sts[b], fill)
    nc.compile()
    return nc


def _prep_inputs(x, W_attn, b_attn, A_attn, B_attn, W_proj, b_proj, A_proj,
                 B_proj):
    xT = np.ascontiguousarray(x.reshape(BT, C).T)
    # Fold LoRA into the dense weights (exact same function, fp32 math).
    W_attn_eff = W_attn + SCALING * (B_attn.astype(np.float64)
                                     @ A_attn.astype(np.float64)
                                     ).astype(np.float32)
    W_proj_eff = W_proj + SCALING * (B_proj.astype(np.float64)
                                     @ A_proj.astype(np.float64)
                                     ).astype(np.float32)
    tri = np.triu(np.ones((128, 128), np.float32))
    eye = np.eye(128, dtype=np.float32)
    in_maps = []
    for c in range(NCORES):
        rows = np.r_[128 * c:128 * c + 128,
                     C + 128 * c:C + 128 * c + 128,
                     2 * C + 128 * c:2 * C + 128 * c + 128]
        W_sl = W_attn_eff[rows]                              # [384, C]
        WqT = np.ascontiguousarray(W_sl.T).reshape(NCIN, 128, 384)
        b_sl = np.ascontiguousarray(b_attn[rows].reshape(3, 128).T)
        ysl = slice(128 * c, 128 * c + 128)
        WpT = np.ascontiguousarray(W_proj_eff[:, ysl].T)     # [128, C]
        h = np.float16
        in_maps.append({
            "xT": xT.astype(h), "Wq": WqT.astype(h), "bq": b_sl,
            "Wp": WpT.astype(h), "tri": tri.astype(h),
            "eye": eye.astype(h),
            "onesb": np.ones((128, 16), h),
            "zerosD": np.zeros((128, 1024), h),
        })
    return in_maps


def _install_ntff_shim():
    """Provide antenv.axon_hooks (missing on this image) via ctypes against
    the axon .so, mirroring trn_agent_boot.trn_boot._ntff_profile_via_ctypes."""
    import types
    import ctypes
    import contextlib
    try:
        from antenv.axon_hooks import get_axon_ntff_profile_hook  # noqa: F401
        return
    except ImportError:
        pass
    so_path = "/opt/axon/libaxon_pjrt.so"
    try:
        lib = ctypes.CDLL(so_path)
    except OSError:
        return
    if not hasattr(lib, "axon_start_nrt_profile"):
        return
    lib.axon_start_nrt_profile.argtypes = [ctypes.POINTER(ctypes.c_int64),
                                           ctypes.c_size_t]
    lib.axon_start_nrt_profile.restype = ctypes.c_int64
    lib.axon_stop_nrt_profile.argtypes = [ctypes.c_char_p]
    lib.axon_stop_nrt_profile.restype = ctypes.c_int64

    @contextlib.contextmanager
    def _hook(output_dir, device_ids):
        import jax
        jax.devices()
        if device_ids:
            ids = (ctypes.c_int64 * len(device_ids))(*device_ids)
            rc = lib.axon_start_nrt_profile(ids, len(device_ids))
        else:
            rc = lib.axon_start_nrt_profile(None, 0)
        if rc != 0:
            raise RuntimeError(f"axon_start_nrt_profile rc={rc}")
        try:
            yield
        finally:
            n = lib.axon_stop_nrt_profile(str(output_dir).encode())
            if n < 0:
                raise RuntimeError(f"axon_stop_nrt_profile rc={n}")

    import antenv
    mod = types.ModuleType("antenv.axon_hooks")
    mod.get_axon_ntff_profile_hook = lambda: _hook
    mod.set_axon_ntff_profile_hook = lambda h: None
    sys.modules["antenv.axon_hooks"] = mod
    antenv.axon_hooks = mod


def run(inputs, trace=False, trace_cores=None):
    """Run the kernel. Returns (output, BassKernelResults)."""
    if "nc" not in _cache:
        _cache["nc"] = _build()
    nc = _cache["nc"]
    inputs = {k: np.asarray(v, dtype=np.float32) for k, v in inputs.items()}
    in_maps = _prep_inputs(**inputs)
    if trace:
        _install_ntff_shim()
    res = run_bass_kernel_spmd(nc, in_maps, core_ids=list(range(NCORES)),
                               trace=trace, trace_cores=trace_cores)
    outT = np.zeros((C, BT), np.float32)
    for r in res.results:
        outT += r["outT"].astype(np.float32)
    out = outT.T + inputs["b_proj"][None, :]
    return out.astype(np.float32).reshape(B, T, C), res


def kernel(**inputs):
    out, _ = run(inputs, trace=False)
    return out
